# revision 3
# baseline (speedup 1.0000x reference)
"""Ragged-segment attention for Trainium2 (8 NeuronCores, SPMD), bin-dense fp16.

V3 over V2: ACT offload (output scaling on DVE/Pool, softmax sums on
DVE), and all three per-group loads merged into one DMA (context, its
transpose, and the mask share one group-blocked input array).

V2 over V1:
- host uploads cpk AND cpkT (both group-blocked, contiguous per partition)
  so the PE does no transposes;
- the low-rank mask matmul is replaced by a full additive mask tile fused
  into the softmax via ONE DVE tensor_tensor_reduce that also produces the
  (negated) row max: scn = -(sc + mask), nmaxn = min(scn) = -max;
  exp then runs as exp(scn * -1 + nmaxn) with the sum accumulated on ACT;
- exp-transpose PSUM->SBUF copies run on the (otherwise idle) Pool engine;
- the padded tail group is narrowed to its real bin count.
"""
import numpy as np

import concourse.bacc as bacc
import concourse.mybir as mybir
import concourse.tile as tile
from concourse.bass_utils import run_bass_kernel_spmd

F32 = mybir.dt.float32
F32R = mybir.dt.float32r
FP16 = mybir.dt.float16

N_CORES = 8
D = 512
BIN = 128
GROUP = 4

LAST_RESULTS = {}


def _plan(lengths, mode):
    S = len(lengths)
    n_slots = S // N_CORES
    order = np.argsort(-lengths, kind="stable")
    seg_ids = [[int(order[N_CORES * j + c]) for j in range(n_slots)]
               for c in range(N_CORES)]
    if mode == "f32r":
        slot_len = [min(128, -(-int(lengths[order[N_CORES * j]]) // 2) * 2)
                    for j in range(n_slots)]
    else:
        slot_len = [int(lengths[order[N_CORES * j]]) for j in range(n_slots)]

    bins = []   # (used-token count, n_segs) per bin
    slots = []  # (bin, off, L)
    for j, L in enumerate(slot_len):
        bi = next((i for i, (used, ns) in enumerate(bins)
                   if used + L <= BIN and ns < 31), None)
        if bi is None:
            bins.append((0, 0))
            bi = len(bins) - 1
        used, ns = bins[bi]
        slots.append((bi, used, L))
        bins[bi] = (used + L, ns + 1)
    n_bins = ((len(bins) + GROUP - 1) // GROUP) * GROUP
    return slots, n_bins, seg_ids


def _mask_layout(slots, n_bins):
    by_bin = [[] for _ in range(n_bins)]
    for bi, off, L in slots:
        by_bin[bi].append((off, L))
    return by_bin


def _build(slots, n_bins, mode, repeat=1, out_fp16=None):
    DT = F32R if mode == "f32r" else FP16
    if out_fp16 is None:
        out_fp16 = (mode == "fp16")
    ODT = FP16 if out_fp16 else F32
    NPDT = np.float32 if mode == "f32r" else np.float16
    nc = bacc.Bacc("TRN2", target_bir_lowering=False)
    n_groups = n_bins // GROUP

    by_bin = _mask_layout(slots, n_bins)
    # real bins per group (trailing groups may be partially padded)
    nb_of = [0] * n_groups
    for g in range(n_groups):
        nb = 0
        for i in range(GROUP):
            if by_bin[g * GROUP + i]:
                nb = i + 1
        nb_of[g] = nb

    # merged per-group input: 4 k-slabs of C^T, then the mask slab, then C
    # (9 slabs of GROUP*128 cols each): one DMA per group loads everything
    cin = nc.dram_tensor("cin", [n_groups * 128, 9 * GROUP * 128], DT,
                         kind="ExternalInput")
    wt = nc.dram_tensor("wt", [128, 4 * D], DT, kind="ExternalInput")
    bvec = nc.dram_tensor("bvec", [128, 4], F32, kind="ExternalInput")
    opk = nc.dram_tensor("opk", [n_groups * 128, GROUP * D], ODT,
                         kind="ExternalOutput")

    ident = nc.inline_tensor(np.eye(128, dtype=NPDT), name="ident")

    with tile.TileContext(nc) as tc:
        with (
            tc.tile_pool(name="const", bufs=1) as cpool,
            tc.tile_pool(name="cb", bufs=4) as cbp,
            tc.tile_pool(name="ctp", bufs=3) as ctp,
            tc.tile_pool(name="utp", bufs=3) as utp,
            tc.tile_pool(name="seg", bufs=12) as segp,
            tc.tile_pool(name="scn", bufs=4) as scnp,
            tc.tile_pool(name="stat", bufs=16) as statp,
            tc.tile_pool(name="outp", bufs=3) as outp,
            tc.tile_pool(name="mk", bufs=3) as mkp,
            tc.tile_pool(name="ups", bufs=2, space="PSUM") as ups,
            tc.tile_pool(name="scps", bufs=2, space="PSUM") as scps,
            tc.tile_pool(name="teps", bufs=2, space="PSUM") as teps,
            tc.tile_pool(name="ops", bufs=2, space="PSUM") as opsp,
        ):
            wt_sb = cpool.tile([128, 4, D], DT, tag="wt")
            b_sb = cpool.tile([128, 4], F32, tag="b")
            id_t = cpool.tile([128, 128], DT, tag="id")
            nc.sync.dma_start(wt_sb[:], wt.ap().rearrange("p (c e) -> p c e", c=4))
            nc.sync.dma_start(b_sb[:], bvec[:])
            nc.sync.dma_start(id_t[:], ident[:] if mode != "f32r"
                              else ident.ap().bitcast(F32R))

            cin_v = cin.ap().rearrange("(g p) (s b t) -> g p s b t",
                                       p=128, s=9, b=GROUP)
            opk_v = opk.ap().rearrange("(g p) (b d) -> g p b d", p=128, b=GROUP)

            def load_group(g):
                nb = nb_of[g]
                cm = cbp.tile([128, 9, GROUP, 128], DT, tag="cm")
                if nb == GROUP:
                    nc.sync.dma_start(cm[:], cin_v[g])
                else:
                    nc.sync.dma_start(cm[:, :, :nb, :], cin_v[g][:, :, :nb, :])
                return cm

            def u_chunk(st, c):
                nb = nb_of[st["g"]]
                cm, ut = st["cm"], st["ut"]
                ups_t = ups.tile([128, GROUP * 128], F32, tag="ups")
                for k in range(4):
                    nc.tensor.matmul(
                        ups_t[:, :nb * 128], wt_sb[:, k, c * 128:(c + 1) * 128],
                        cm[:, k, :nb, :], start=(k == 0), stop=(k == 3))
                nc.scalar.activation(
                    ut[:, c, :nb, :], ups_t[:, :nb * 128],
                    mybir.ActivationFunctionType.Tanh, bias=b_sb[:, c:c + 1])

            def bin_scores(st, i):
                g = st["g"]
                if not by_bin[g * GROUP + i]:
                    return
                cm, ut = st["cm"], st["ut"]
                sc = scps.tile([128, 128], F32, tag="sc")
                for k in range(4):
                    nc.tensor.matmul(
                        sc[:], cm[:, k, i, :], ut[:, k, i, :],
                        start=(k == 0), stop=(k == 3))
                # fused mask-add + negated-row-max in one DVE pass:
                #   scn = -(sc + mask);  nmaxn = min(scn) = -max(sc + mask)
                scn = scnp.tile([128, 128], F32, tag="scn")
                nmaxn = statp.tile([128, 1], F32, tag="nmax")
                sums = statp.tile([128, 1], F32, tag="sums")
                recip = statp.tile([128, 1], F32, tag="recip")
                expt = segp.tile([128, 128], DT, tag="expt")
                nc.vector.tensor_add(scn[:], sc[:], cm[:, 4, i, :])
                nc.vector.tensor_reduce(
                    nmaxn[:], scn[:], axis=mybir.AxisListType.X,
                    op=mybir.AluOpType.max, negate=True)
                nc.scalar.activation(
                    expt[:], scn[:], mybir.ActivationFunctionType.Exp,
                    bias=nmaxn[:], accum_out=sums[:])
                nc.vector.reciprocal(recip[:], sums[:])
                st[("bin", i)] = (expt, recip)

            def bin_expT(st, i):
                if ("bin", i) not in st:
                    return
                expt, recip = st[("bin", i)]
                tp = teps.tile([128, 128], DT, tag="te")
                nc.tensor.transpose(tp[:], expt[:], id_t[:])
                attn = segp.tile([128, 128], DT, tag="attn")
                nc.vector.tensor_copy(attn[:], tp[:])
                st[("attn", i)] = (attn, recip)

            def bin_out(st, i):
                if ("attn", i) not in st:
                    return
                attn, recip = st.pop(("attn", i))
                st.pop(("bin", i))
                cm, og = st["cm"], st["og"]
                ops_t = opsp.tile([128, D], F32, tag="ops")
                nc.tensor.matmul(ops_t[:], attn[:], cm[:, 5:9, i, :],
                                 start=True, stop=True)
                if i == 3:
                    nc.scalar.activation(og[:, i, :], ops_t[:],
                                         mybir.ActivationFunctionType.Copy,
                                         scale=recip[:])
                else:
                    nc.vector.tensor_scalar_mul(og[:, i, :], ops_t[:], recip[:])

            def store_group(st):
                g = st["g"]
                nb = nb_of[g]
                nc.scalar.dma_start(opk_v[g][:, :nb, :], st["og"][:, :nb, :])

            niter = repeat * n_groups
            states = {}
            for it in range(niter + 2):
                if it < niter:
                    g = it % n_groups
                    cm = load_group(g)
                    ut_t = utp.tile([128, 4, GROUP, 128], DT, tag="ut")
                    og_t = outp.tile([128, GROUP, D], ODT, tag="og")
                    st_new = {"g": g, "cm": cm, "ut": ut_t, "og": og_t}
                else:
                    st_new = None
                st_mid = states.get(it - 1)
                st_old = states.pop(it - 2, None)

                if st_new is not None:
                    u_chunk(st_new, 0)
                if st_mid is not None:
                    bin_scores(st_mid, 0)
                    bin_scores(st_mid, 1)
                if st_new is not None:
                    u_chunk(st_new, 1)
                if st_mid is not None:
                    bin_scores(st_mid, 2)
                    bin_scores(st_mid, 3)
                if st_new is not None:
                    u_chunk(st_new, 2)
                if st_mid is not None:
                    for i in range(GROUP):
                        bin_expT(st_mid, i)
                if st_new is not None:
                    u_chunk(st_new, 3)
                if st_old is not None:
                    for i in range(GROUP):
                        bin_out(st_old, i)
                    store_group(st_old)
                if st_new is not None:
                    states[it] = st_new

    nc.compile()
    return nc


def _host_arrays(slots, n_bins, seg_ids, lengths, context, W, b, mode,
                 out_fp16=None):
    DT = np.float32 if mode == "f32r" else np.float16
    NEG = -1.0e30 if mode == "f32r" else -30000.0
    T = n_bins * BIN
    by_bin2 = [[] for _ in range(n_bins)]
    for j, (bi, off, L) in enumerate(slots):
        by_bin2[bi].append((j, off, L))
    n_groups = n_bins // GROUP

    wt = np.ascontiguousarray(
        W.T.reshape(4, 128, D).transpose(1, 0, 2).reshape(128, 4 * D)).astype(DT)
    bvec = np.ascontiguousarray(b.reshape(4, 128).T).astype(np.float32)

    in_maps = []
    for c in range(N_CORES):
        # full additive mask per bin, using THIS core's true segment lengths
        # for the key extent (slots are padded to the slot-max length)
        bmask = np.full((n_bins, BIN, BIN), NEG, np.float32)
        for bb in range(n_bins):
            for j, off, L in by_bin2[bb]:
                n = int(lengths[seg_ids[c][j]])
                bmask[bb, off:off + L, off:off + n] = 0.0
        msk_blk = bmask.reshape(n_groups, GROUP, 128, 128).transpose(
            0, 2, 1, 3).astype(DT)                      # [g, q, b, k]
        cpk = np.zeros((T, D), DT)
        for j, (bi, off, _L) in enumerate(slots):
            s = seg_ids[c][j]
            n = int(lengths[s])
            r0 = bi * BIN + off
            cpk[r0:r0 + n] = context[s, :n].astype(DT)
        A = cpk.reshape(n_groups, GROUP, 128, 4, 128)
        x1 = A.transpose(0, 4, 3, 1, 2)                 # [g, dl, k, b, t]
        x2 = A.transpose(0, 2, 3, 1, 4)                 # [g, t, k, b, dl]
        cin = np.concatenate([x1, msk_blk[:, :, None], x2], axis=2)
        cin = np.ascontiguousarray(cin).reshape(n_groups * 128, 9 * GROUP * 128)
        in_maps.append({"cin": cin, "wt": wt, "bvec": bvec})
    return in_maps


_CACHE = {}


def kernel(context, lengths, W, b, mode="fp16"):
    context = np.asarray(context, dtype=np.float32)
    lengths = np.asarray(lengths, dtype=np.int32)
    W = np.asarray(W, dtype=np.float32)
    b = np.asarray(b, dtype=np.float32)
    S, Lmax, Din = context.shape

    slots, n_bins, seg_ids = _plan(lengths, mode)
    key = (tuple(slots), n_bins, mode)
    if key in _CACHE:
        nc = _CACHE[key]
    else:
        nc = _build(slots, n_bins, mode)
        _CACHE[key] = nc

    in_maps = _host_arrays(slots, n_bins, seg_ids, lengths, context, W, b, mode)
    res = run_bass_kernel_spmd(nc, in_maps, list(range(N_CORES)))
    LAST_RESULTS["exec_time_ns"] = res.exec_time_ns

    n_groups = n_bins // GROUP
    out = np.zeros((S, Lmax, D), np.float32)
    for c in range(N_CORES):
        opk_b = res.results[c]["opk"].astype(np.float32)
        opk = opk_b.reshape(n_groups, 128, GROUP, 4, 128).transpose(
            0, 2, 1, 3, 4).reshape(n_bins * BIN, D)
        for j, (bi, off, _L) in enumerate(slots):
            s = seg_ids[c][j]
            n = int(lengths[s])
            r0 = bi * BIN + off
            out[s, :n] = opk[r0:r0 + n]
    return out
